# revision 1
# baseline (speedup 1.0000x reference)
"""Trainium2 Bass kernel for nn_CompetitiveNetwork (competitive-binding solve).

Math (per batch row b):
    K  = clip(exp(K_raw), 0, 1e3)   BT = clip(exp(BT_raw), 0, 1e3)
    iterate 21x:  BF' = 1/(1 + K^T AF);  AF = AT * 1/(1 + (K*diag(BT)) BF')
    final:        BF' = 1/(1 + K^T AF)
    Y = AF^T (K * clip(W) * BT) BF' + b     (bilinear; (B,4096) C never built)

Sharding: pure data-parallel over batch (16384 -> 8 cores x 2048).
Device layout: transposed state (features on partitions, batch on free),
two 64-partition streams stacked into (128, FD) tiles; 4 column chunks,
group-staggered by one half-step for steady engine pipelining.

Precision plan (errors are damped by the fixed point's ~0.6/iter
contraction; validated numerically against the fp32 reference):
  - early iters: fp16 matmuls (1 cyc/row on PE) + fp16 state,
    reciprocals on ACT (raw Reciprocal, +1 via bias, ~1.2e-5) or on DVE
    via a custom 1-instruction NEWTON1P refinement of the previous
    iterate; AF multiplies split DVE (fp16 2x mode) / GPSIMD.
  - last FP32_TAIL_ITERS iterations + readout: exact fp32 matmuls,
    Newton reciprocals (error = drift^2, i.e. ~exact at the tail).
"""

import numpy as np

import concourse.bacc as bacc
import concourse.mybir as mybir
from concourse.tile import TileContext
from concourse.bass_utils import run_bass_kernel_spmd


# --- custom DVE op: NEWTON1P_ANT (inlined for self-containment) ---
# out = (c1 - (in0 + c0) * in1) * in1 : one Newton-Raphson refinement
# of in1 toward 1/(1+in0) in a single 4-stage DVE instruction.

import numpy as np

import concourse.dve_ops as dve_ops
from concourse.dve_ops import DveOp
from concourse.dve_spec import Spec, Src0, Src1, C0, C1, lower


def _ref_newton1p(in0, in1, c0, c1, c2):
    return ((c1 - (in0.astype(np.float32) + c0) * in1) * in1).astype(np.float32)


def _make_op(shas):
    return DveOp(
        "NEWTON1P_ANT",
        Spec(
            body=(C1 - (Src0 + C0) * Src1) * Src1,
            reference=_ref_newton1p,
        ),
        subdim=False,
        uops_sha=shas,
    )


def register():
    for op in dve_ops.OPS:
        if op.name == "NEWTON1P_ANT":
            return op
    # compute shas by lowering once with the opcode the registry will assign
    probe = _make_op({})
    opcode = dve_ops._CUSTOM_DVE_ROW_BASE + len(dve_ops.OPS)
    shas = {}
    for ver in ("v3", "v4"):
        try:
            from concourse.dve_uop import DveOpSpec
            res = DveOpSpec(name=probe.name, opcode=opcode,
                            uops=lower(probe.spec, ver=ver),
                            rd1_en=True)
            shas[ver] = res.sha(ver)
        except Exception as e:
            print(f"lower {ver} failed: {e}")
    op = _make_op(shas)
    dve_ops.OPS.append(op)
    dve_ops.CUSTOM_DVE_SPECS[op.name] = op.spec
    dve_ops._SUB_OPCODE_FOR_NAME[op.name] = (
        dve_ops._CUSTOM_DVE_ROW_BASE + len(dve_ops.OPS) - 1)
    return op


def newton1p(nc_vector, out, in0, in1):
    """out = (2 - (in0 + 1) * in1) * in1 on the DVE."""
    op = register()
    return nc_vector._custom_dve(op, out=out, in0=in0, in1=in1,
                                 s0=1.0, s1=2.0, imm2=0.0)



class newton_op:  # namespace shim matching the former module
    register = staticmethod(register)
    newton1p = staticmethod(newton1p)

B, NA, NB = 16384, 64, 64
N_CORES = 8
B_CORE = B // N_CORES          # 2048 batch rows per core
N_CHUNK = 4
FD = B_CORE // 2 // N_CHUNK    # 256
N_FULL_ITERS = 21              # 20 fori iters + refinement (BF,AF)
ACT_ONLY_ITERS = 3             # ACT recips until Newton seeds are usable
FP32_TAIL_ITERS = 5            # exact-fp32 iterations at the end

FP32 = mybir.dt.float32
FP16 = mybir.dt.float16

# per-op engine costs (ns) at FD=256 for the greedy balancer
_COST_ACT_RECIP = 398.0
_COST_DVE_RECIP = 392.0
_COST_DVE_MUL = 194.0          # fp16 2x mode
_COST_GPS_MUL = 450.0

_CACHE = {}


class _Balancer:
    def __init__(self):
        self.load = {"act": 0.0, "dve": 0.0, "gps": 0.0}

    def recip(self, n):
        if n < ACT_ONLY_ITERS:
            self.load["act"] += _COST_ACT_RECIP
            return "act"
        if n >= N_FULL_ITERS - FP32_TAIL_ITERS:
            self.load["dve"] += _COST_DVE_RECIP   # Newton tail: near exact
            return "dve"
        if self.load["act"] + _COST_ACT_RECIP <= self.load["dve"] + _COST_DVE_RECIP:
            self.load["act"] += _COST_ACT_RECIP
            return "act"
        self.load["dve"] += _COST_DVE_RECIP
        return "dve"

    def mul(self, dve_cost):
        if self.load["dve"] + dve_cost <= self.load["gps"] + _COST_GPS_MUL:
            self.load["dve"] += dve_cost
            return "dve"
        self.load["gps"] += _COST_GPS_MUL
        return "gps"


def _act_recip(nc, out_ap, in_ap):
    eng = nc.scalar
    ins = [eng.lower_ap(in_ap),
           mybir.ImmediateValue(dtype=FP32, value=1.0),   # bias: +1
           mybir.ImmediateValue(dtype=FP32, value=1.0),   # scale
           mybir.ImmediateValue(dtype=FP32, value=0.0)]   # alpha
    eng.add_instruction(mybir.InstActivation(
        name=nc.get_next_instruction_name(),
        func=mybir.ActivationFunctionType.Reciprocal,
        ins=ins, outs=[eng.lower_ap(out_ap)]))


def _build_module(repeat=1):
    newton_op.register()
    nc = bacc.Bacc()
    att = nc.dram_tensor("att", (128, N_CHUNK * FD), FP32, kind="ExternalInput")
    w1 = nc.dram_tensor("w1", (64, 64), FP32, kind="ExternalInput")
    w2 = nc.dram_tensor("w2", (64, 64), FP32, kind="ExternalInput")
    m2 = nc.dram_tensor("m2", (64, 64), FP32, kind="ExternalInput")
    yout = nc.dram_tensor("yout", (2 * N_CHUNK, FD), FP32, kind="ExternalOutput")

    def mm_pair(psum, lhsT128, rhs128):
        """Two quadrant matmuls: lower (rows 0:64) and upper (64:128) streams."""
        nc.tensor.matmul(out=psum[0:64, :], lhsT=lhsT128[0:64, :],
                         rhs=rhs128[0:64, :], start=True, stop=True)
        nc.tensor.matmul(out=psum[64:128, :], lhsT=lhsT128[64:128, :],
                         rhs=rhs128[64:128, :], start=True, stop=True)

    with TileContext(nc) as tc, \
         tc.tile_pool(name="const", bufs=1) as cpool, \
         tc.tile_pool(name="state", bufs=2) as spool, \
         tc.tile_pool(name="work", bufs=3) as wpool, \
         tc.tile_pool(name="psum", bufs=8, space="PSUM") as ppool:

        w1f = cpool.tile([128, 64], FP32, tag="w1f")
        w2f = cpool.tile([128, 64], FP32, tag="w2f")
        m2f = cpool.tile([128, 64], FP32, tag="m2f")
        for dst, src in ((w1f, w1), (w2f, w2), (m2f, m2)):
            nc.sync.dma_start(out=dst[0:64, :], in_=src[:, :])
            nc.sync.dma_start(out=dst[64:128, :], in_=src[:, :])
        w1h = cpool.tile([128, 64], FP16, tag="w1h")
        w2h = cpool.tile([128, 64], FP16, tag="w2h")
        nc.vector.tensor_copy(w1h[:], w1f[:])
        nc.vector.tensor_copy(w2h[:], w2f[:])
        ones = cpool.tile([128, 1], FP32, tag="ones")
        nc.vector.memset(ones[:], 1.0)

        ats, ats16 = [], []
        for c in range(N_CHUNK):
            at_c = cpool.tile([128, FD], FP32, tag=f"at{c}")
            nc.sync.dma_start(out=at_c[:], in_=att[:, c * FD:(c + 1) * FD])
            ats.append(at_c)
            a16 = cpool.tile([128, FD], FP16, tag=f"ath{c}")
            nc.vector.tensor_copy(a16[:], at_c[:])
            ats16.append(a16)

        for _rep in range(repeat):
            af = list(ats16)
            bf = [None] * N_CHUNK
            rr = [None] * N_CHUNK    # previous AF-step reciprocal (Newton seed)
            bal = _Balancer()

            def recip(ps, out_tile, seed_tile, engine):
                if engine == "act":
                    _act_recip(nc, out_tile[:], ps[:])
                else:
                    newton_op.newton1p(nc.vector, out_tile[:], ps[:], seed_tile[:])

            def emit_halfstep(c, h):
                n = h // 2
                # fp16 era while the *input* state is fp16: the BF half-step
                # at the boundary iteration still consumes fp16 state
                fp16_mm = n < N_FULL_ITERS - FP32_TAIL_ITERS or (
                    h % 2 == 0 and n == N_FULL_ITERS - FP32_TAIL_ITERS)
                st_dt = FP16 if n < N_FULL_ITERS - FP32_TAIL_ITERS else FP32
                if h % 2 == 0:
                    # S = K^T AF ; BF' = 1/(1+S)
                    ps = ppool.tile([128, FD], FP32, tag="ps")
                    mm_pair(ps, w1h if fp16_mm else w1f, af[c])
                    bf_n = spool.tile([128, FD], st_dt, tag=f"bf{c}")
                    recip(ps, bf_n, bf[c], bal.recip(n))
                    bf[c] = bf_n
                else:
                    # T = (K*BT) BF' ; AF = AT / (1+T)
                    ps2 = ppool.tile([128, FD], FP32, tag="ps")
                    mm_pair(ps2, w2h if fp16_mm else w2f, bf[c])
                    r_n = spool.tile([128, FD], st_dt, tag=f"r{c}")
                    recip(ps2, r_n, rr[c], bal.recip(n))
                    rr[c] = r_n
                    af_n = spool.tile([128, FD], st_dt, tag=f"af{c}")
                    at_src = ats16[c] if st_dt == FP16 else ats[c]
                    mul_cost = _COST_DVE_MUL if st_dt == FP16 else 327.0
                    if bal.mul(mul_cost) == "dve":
                        nc.vector.tensor_mul(af_n[:], at_src[:], r_n[:])
                    else:
                        nc.gpsimd.tensor_mul(af_n[:], at_src[:], r_n[:])
                    af[c] = af_n

            # group B (chunks 2,3) one half-step behind group A (0,1): each
            # tick mixes BF- and AF-type work so every engine's in-order
            # stream has a steady supply of ready instructions.
            H = 2 * N_FULL_ITERS
            for t in range(H + 1):
                for c in (0, 1):
                    if t < H:
                        emit_halfstep(c, t)
                for c in (2, 3):
                    if t >= 1:
                        emit_halfstep(c, t - 1)

            # final BF' + bilinear readout (all exact fp32)
            pss, gps, bfs, hs = [], [], [], []
            for c in range(N_CHUNK):
                ps = ppool.tile([128, FD], FP32, tag="ps")
                mm_pair(ps, w1f, af[c])
                pss.append(ps)
                gp = ppool.tile([128, FD], FP32, tag="ps")
                mm_pair(gp, m2f, af[c])
                gps.append(gp)
            for c in range(N_CHUNK):
                bf_f = spool.tile([128, FD], FP32, tag=f"bf{c}")
                newton_op.newton1p(nc.vector, bf_f[:], pss[c][:], bf[c][:])
                bfs.append(bf_f)
            for c in range(N_CHUNK):
                h = wpool.tile([128, FD], FP32, tag="h")
                nc.vector.tensor_mul(h[:], gps[c][:], bfs[c][:])
                hs.append(h)
            for c in range(N_CHUNK):
                yp = ppool.tile([128, FD], FP32, tag="ps")
                nc.tensor.matmul(out=yp[0:1, :], lhsT=ones[0:64, :],
                                 rhs=hs[c][0:64, :], start=True, stop=True)
                nc.tensor.matmul(out=yp[64:65, :], lhsT=ones[64:128, :],
                                 rhs=hs[c][64:128, :], start=True, stop=True)
                ys = wpool.tile([128, FD], FP32, tag="ys")
                nc.scalar.copy(ys[0:1, :], yp[0:1, :])
                nc.scalar.copy(ys[64:65, :], yp[64:65, :])
                nc.sync.dma_start(out=yout[c:c + 1, :], in_=ys[0:1, :])
                nc.sync.dma_start(out=yout[N_CHUNK + c:N_CHUNK + c + 1, :],
                                  in_=ys[64:65, :])

    nc.finalize()
    return nc


def _get_module(repeat=1):
    key = f"nc{repeat}"
    if key not in _CACHE:
        _CACHE[key] = _build_module(repeat)
    return _CACHE[key]


def kernel(AT, K_raw, BT_raw, W_raw, b_raw, _run_kw=None, _repeat=1):
    AT = np.asarray(AT, dtype=np.float32)
    K = np.clip(np.exp(np.asarray(K_raw, np.float32)), 0.0, 1000.0).astype(np.float32)
    BT = np.clip(np.exp(np.asarray(BT_raw, np.float32)), 0.0, 1000.0).astype(np.float32)
    Wc = np.clip(np.asarray(W_raw, np.float32), -10.0, 10.0).reshape(NA, NB)
    b0 = np.clip(np.asarray(b_raw, np.float32), -10.0, 10.0)[0]

    w1 = np.ascontiguousarray(K)                       # lhsT for S = K^T AF
    w2 = np.ascontiguousarray((K * BT[None, :]).T)     # lhsT for T = K' BF'
    m2 = np.ascontiguousarray(K * Wc * BT[None, :])    # bilinear weights

    att = np.ascontiguousarray(AT.T)                   # (64, 16384)

    in_maps = []
    for c in range(N_CORES):
        chunk = att[:, c * B_CORE:(c + 1) * B_CORE]    # (64, 2048)
        stacked = np.ascontiguousarray(
            np.concatenate([chunk[:, :B_CORE // 2], chunk[:, B_CORE // 2:]], axis=0))
        in_maps.append({"att": stacked, "w1": w1, "w2": w2, "m2": m2})

    nc = _get_module(_repeat)
    res = run_bass_kernel_spmd(nc, in_maps, core_ids=list(range(N_CORES)),
                               **(_run_kw or {}))
    out = np.empty((B,), np.float32)
    for c in range(N_CORES):
        out[c * B_CORE:(c + 1) * B_CORE] = res.results[c]["yout"].reshape(-1)
    if _run_kw is not None:
        _CACHE["last_result"] = res
    return out + b0



# revision 3
# speedup vs baseline: 2.4580x; 2.4580x over previous
"""Trainium2 Bass kernel for nn_CompetitiveNetwork (competitive-binding solve).

Math (per batch column):
    K  = clip(exp(K_raw), 0, 1e3)   BT = clip(exp(BT_raw), 0, 1e3)
    fixed point:  u = 1/(1 + K^T AF);  AF = AT / (1 + (K diag(BT)) u)
    readout:      Y = sum_b (M^T AF)_b * u_b + b,  M = K*W*BT

Device algorithm (accelerated, validated numerically on host):
  state G (gain; AF = AT*G) and u. Per iteration:
    S  = W1blk @ af          (fp32r matmul, 128-part block-diag = 2 streams)
    u  = 1/(1+S)             (ACT reciprocal, exact)
    Tw = wg*W2blk @ u        (fp32r matmul; wg folded into weights)
    G  = ((1+wg) - (Tw+wg))*G)*G   one fused DVE op == SOR(wg) + Newton
         (iters 1..E_G use the exact path: ACT recip + scalar_tensor_tensor)
    af = AT*G                (DVE/Pool mul)
  9 over-relaxed iterations (wg=1.4) replace the reference's 21.5 plain
  iterations: SOR contraction ~0.33/iter vs 0.6, landing ~1.8e-3 from the
  reference iterate (fixed-point limit itself is only ~9e-5 away).
  All matmuls use fp32r (1 cyc/row at FD>=256; measured 9e-5 accurate on hw).

Sharding: pure data-parallel over batch (16384 -> 8 cores x 2048).
Layout: features on partitions, batch on free dim; two 64-partition streams
stacked into (128, FD) tiles; 4 column chunks of FD=256.
"""

import numpy as np

import concourse.bacc as bacc
import concourse.mybir as mybir
from concourse.tile import TileContext
from concourse.bass_utils import run_bass_kernel_spmd


# --- custom DVE op: NEWTON1P_ANT ---
# out = (c1 - (in0 + c0) * in1) * in1 : with (c0,c1)=(1,2) one Newton step
# of in1 toward 1/(1+in0); with (c0,c1)=(w,1+w) and in0=w*T it fuses the
# Newton step with SOR mixing: out = (1-w)*in1 + w*newton(in1; 1+T).

import concourse.dve_ops as dve_ops
from concourse.dve_ops import DveOp
from concourse.dve_spec import Spec, Src0, Src1, C0, C1, lower


def _ref_newton1p(in0, in1, c0, c1, c2):
    return ((c1 - (in0.astype(np.float32) + c0) * in1) * in1).astype(np.float32)


def _make_op(shas):
    return DveOp(
        "NEWTON1P_ANT",
        Spec(
            body=(C1 - (Src0 + C0) * Src1) * Src1,
            reference=_ref_newton1p,
        ),
        subdim=False,
        uops_sha=shas,
    )


def register():
    for op in dve_ops.OPS:
        if op.name == "NEWTON1P_ANT":
            return op
    probe = _make_op({})
    opcode = dve_ops._CUSTOM_DVE_ROW_BASE + len(dve_ops.OPS)
    shas = {}
    for ver in ("v3", "v4"):
        try:
            from concourse.dve_uop import DveOpSpec
            res = DveOpSpec(name=probe.name, opcode=opcode,
                            uops=lower(probe.spec, ver=ver),
                            rd1_en=True)
            shas[ver] = res.sha(ver)
        except Exception as e:
            print(f"lower {ver} failed: {e}")
    op = _make_op(shas)
    dve_ops.OPS.append(op)
    dve_ops.CUSTOM_DVE_SPECS[op.name] = op.spec
    dve_ops._SUB_OPCODE_FOR_NAME[op.name] = (
        dve_ops._CUSTOM_DVE_ROW_BASE + len(dve_ops.OPS) - 1)
    return op


def newton_sor(nc_vector, out, in0, in1, c0, c1):
    """out = (c1 - (in0 + c0) * in1) * in1 on the DVE."""
    op = register()
    return nc_vector._custom_dve(op, out=out, in0=in0, in1=in1,
                                 s0=float(c0), s1=float(c1), imm2=0.0)


B, NA, NB = 16384, 64, 64
N_CORES = 8
B_CORE = B // N_CORES          # 2048 batch columns per core
N_CHUNK = 4
FD = B_CORE // 2 // N_CHUNK    # 256

N_ITERS = 9                    # over-relaxed iterations
WG = 1.4                       # SOR factor on the G (gain) update
E_G = 3                        # iters 1..E_G use the exact ACT+stt G path

FP32 = mybir.dt.float32
FP32R = mybir.dt.float32r

_CACHE = {}


def _act_recip(nc, out_ap, in_ap, bias=1.0, scale=1.0):
    """out = 1/(in*scale + bias) on the Activation engine."""
    eng = nc.scalar
    ins = [eng.lower_ap(in_ap),
           mybir.ImmediateValue(dtype=FP32, value=float(bias)),
           mybir.ImmediateValue(dtype=FP32, value=float(scale)),
           mybir.ImmediateValue(dtype=FP32, value=0.0)]
    eng.add_instruction(mybir.InstActivation(
        name=nc.get_next_instruction_name(),
        func=mybir.ActivationFunctionType.Reciprocal,
        ins=ins, outs=[eng.lower_ap(out_ap)]))


def _build_module(repeat=1):
    register()
    nc = bacc.Bacc()
    att = nc.dram_tensor("att", (128, N_CHUNK * FD), FP32, kind="ExternalInput")
    # weights: [W1blk | W2a | W2b | M2blk | ones2]  (128, 514)
    wts = nc.dram_tensor("wts", (128, 4 * 128 + 2), FP32, kind="ExternalInput")
    yout = nc.dram_tensor("yout", (2 * N_CHUNK, FD), FP32, kind="ExternalOutput")

    with TileContext(nc) as tc, \
         tc.tile_pool(name="const", bufs=1) as cpool, \
         tc.tile_pool(name="state", bufs=2) as spool, \
         tc.tile_pool(name="work", bufs=3) as wpool, \
         tc.tile_pool(name="psum", bufs=8, space="PSUM") as ppool:

        wall = cpool.tile([128, 4 * 128 + 2], FP32, tag="wall")
        nc.sync.dma_start(out=wall[:], in_=wts[:, :])
        wallr = cpool.tile([128, 4 * 128 + 2], FP32R, tag="wallr")
        nc.vector.tensor_copy(wallr[:], wall[:])
        w1r = wallr[:, 0:128]
        w2ar = wallr[:, 128:256]
        w2br = wallr[:, 256:384]
        m2r = wallr[:, 384:512]
        onesr = wallr[:, 512:514]

        ats, atrs = [], []
        for c in range(N_CHUNK):
            at_c = cpool.tile([128, FD], FP32, tag=f"at{c}")
            nc.sync.dma_start(out=at_c[:], in_=att[:, c * FD:(c + 1) * FD])
            ats.append(at_c)
            atr_c = cpool.tile([128, FD], FP32R, tag=f"atr{c}")
            nc.vector.tensor_copy(atr_c[:], at_c[:])
            atrs.append(atr_c)

        for _rep in range(repeat):
            af = list(atrs)             # AF_0 = AT (G_0 = 1)
            us = [None] * N_CHUNK
            gs = [None] * N_CHUNK

            for n in range(1, N_ITERS + 1):
                pss, ps2s = [], []
                for c in range(N_CHUNK):
                    ps = ppool.tile([128, FD], FP32, tag="ps")
                    nc.tensor.matmul(out=ps[:], lhsT=w1r, rhs=af[c][:],
                                     start=True, stop=True)
                    pss.append(ps)
                for c in range(N_CHUNK):
                    u_n = spool.tile([128, FD], FP32R, tag=f"u{c}")
                    _act_recip(nc, u_n[:], pss[c][:])        # u = 1/(1+S)
                    us[c] = u_n
                for c in range(N_CHUNK):
                    ps2 = ppool.tile([128, FD], FP32, tag="ps")
                    nc.tensor.matmul(out=ps2[:], lhsT=(w2ar if n == 1 else w2br),
                                     rhs=us[c][:], start=True, stop=True)
                    ps2s.append(ps2)
                for c in range(N_CHUNK):
                    g_n = spool.tile([128, FD], FP32, tag=f"g{c}")
                    if n == 1:
                        # G_1 = 1/(1+T)   (wg=1 first step)
                        _act_recip(nc, g_n[:], ps2s[c][:])
                    elif n <= E_G:
                        # exact SOR: R = wg/(1+T) via ACT on ps2 = wg*T;
                        # G = (1-wg)*G + wg*R ... folded: R'=1/(ps2/wg^2+1/wg)
                        # = wg/(1+T); G = G*(1-wg) + R'*wg -> fold wg into R':
                        # use scale=1/wg^2 * wg = 1/wg? Compute R = wg/(1+T)
                        # directly then stt with scalar (1-wg), op1 add needs
                        # R*wg... instead: R2 = wg^2/(wg*T+wg) = wg/(1+T):
                        r_n = wpool.tile([128, FD], FP32, tag=f"r{c}")
                        _act_recip(nc, r_n[:], ps2s[c][:],
                                   bias=1.0 / WG, scale=1.0 / (WG * WG))
                        # r_n = (1/wg)/(1+T); want G=(1-wg)G + wg/(1+T)
                        #   = (G*(1-wg)) + r_n*wg^2 ... stt only scales in0.
                        # So compute with in0 = r_n: (r_n * wg^2) + ... op1
                        # add in1 needs G*(1-wg) -- not expressible in one stt.
                        # Use scale so ACT emits wg/(1+T) directly:
                        #   1/(ps2*s + b) = wg/(1+T) when s=1/wg^2, b=1/wg
                        #   gives 1/( T/wg + 1/wg ) = wg/(1+T). (r_n above IS
                        #   wg/(1+T); comment chain kept for the derivation.)
                        nc.vector.scalar_tensor_tensor(
                            out=g_n[:], in0=gs[c][:], scalar=float(1.0 - WG),
                            in1=r_n[:], op0=mybir.AluOpType.mult,
                            op1=mybir.AluOpType.add)
                    else:
                        # fused Newton+SOR: G = ((1+wg)-(ps2+wg)*G)*G
                        newton_sor(nc.vector, g_n[:], ps2s[c][:], gs[c][:],
                                   WG, 1.0 + WG)
                    gs[c] = g_n
                for c in range(N_CHUNK):
                    af_n = spool.tile([128, FD], FP32R, tag=f"af{c}")
                    if c < 2:
                        nc.vector.tensor_mul(af_n[:], ats[c][:], gs[c][:])
                    else:
                        nc.gpsimd.tensor_mul(af_n[:], ats[c][:], gs[c][:])
                    af[c] = af_n

            # readout: S = W1 af; u = newton(u); V = M2 af; h = V*u; Y = ones^T h
            pss, pps = [], []
            for c in range(N_CHUNK):
                ps = ppool.tile([128, FD], FP32, tag="ps")
                nc.tensor.matmul(out=ps[:], lhsT=w1r, rhs=af[c][:],
                                 start=True, stop=True)
                pss.append(ps)
                pp = ppool.tile([128, FD], FP32, tag="ps")
                nc.tensor.matmul(out=pp[:], lhsT=m2r, rhs=af[c][:],
                                 start=True, stop=True)
                pps.append(pp)
            ufs = []
            for c in range(N_CHUNK):
                u1 = spool.tile([128, FD], FP32, tag=f"u{c}")
                newton_sor(nc.vector, u1[:], pss[c][:], us[c][:], 1.0, 2.0)
                u2 = spool.tile([128, FD], FP32, tag=f"uf{c}")
                newton_sor(nc.vector, u2[:], pss[c][:], u1[:], 1.0, 2.0)
                ufs.append(u2)
            hs = []
            for c in range(N_CHUNK):
                h = wpool.tile([128, FD], FP32R, tag="h")
                nc.vector.tensor_mul(h[:], pps[c][:], ufs[c][:])
                hs.append(h)
            for c in range(N_CHUNK):
                yp = ppool.tile([128, FD], FP32, tag="ps")
                nc.tensor.matmul(out=yp[0:2, :], lhsT=onesr,
                                 rhs=hs[c][:], start=True, stop=True)
                ys = wpool.tile([128, FD], FP32, tag="ys")
                nc.scalar.copy(ys[0:2, :], yp[0:2, :])
                nc.sync.dma_start(out=yout[2 * c:2 * c + 2, :], in_=ys[0:2, :])

    nc.finalize()
    return nc


def _get_module(repeat=1):
    key = f"nc{repeat}"
    if key not in _CACHE:
        _CACHE[key] = _build_module(repeat)
    return _CACHE[key]


def kernel(AT, K_raw, BT_raw, W_raw, b_raw, _run_kw=None, _repeat=1):
    AT = np.asarray(AT, dtype=np.float32)
    K = np.clip(np.exp(np.asarray(K_raw, np.float32)), 0.0, 1000.0).astype(np.float32)
    BT = np.clip(np.exp(np.asarray(BT_raw, np.float32)), 0.0, 1000.0).astype(np.float32)
    Wc = np.clip(np.asarray(W_raw, np.float32), -10.0, 10.0).reshape(NA, NB)
    b0 = np.clip(np.asarray(b_raw, np.float32), -10.0, 10.0)[0]

    w2 = np.ascontiguousarray((K * BT[None, :]).T)     # (nB,nA) lhsT: T = w2^T u
    M = K * Wc * BT[None, :]                           # (nA,nB) lhsT: V = M^T af

    def blk(a):
        z = np.zeros((128, 128), np.float32)
        z[0:64, 0:64] = a
        z[64:128, 64:128] = a
        return z

    ones2 = np.zeros((128, 2), np.float32)
    ones2[0:64, 0] = 1.0
    ones2[64:128, 1] = 1.0
    wts = np.ascontiguousarray(np.concatenate(
        [blk(K), blk(w2), blk(WG * w2), blk(M), ones2], axis=1))

    att = np.ascontiguousarray(AT.T)                   # (64, 16384)
    HB = B_CORE // 2                                   # 1024 cols per stream
    in_maps = []
    for c in range(N_CORES):
        chunk = att[:, c * B_CORE:(c + 1) * B_CORE]    # (64, 2048)
        stacked = np.ascontiguousarray(
            np.concatenate([chunk[:, :HB], chunk[:, HB:]], axis=0))
        in_maps.append({"att": stacked, "wts": wts})

    nc = _get_module(_repeat)
    res = run_bass_kernel_spmd(nc, in_maps, core_ids=list(range(N_CORES)),
                               **(_run_kw or {}))
    out = np.empty((B,), np.float32)
    for co in range(N_CORES):
        y = res.results[co]["yout"]                    # (8, 256)
        base = co * B_CORE
        for c in range(N_CHUNK):
            out[base + c * FD:base + (c + 1) * FD] = y[2 * c]
            out[base + HB + c * FD:base + HB + (c + 1) * FD] = y[2 * c + 1]
    if _run_kw is not None:
        _CACHE["last_result"] = res
    return out + b0


# revision 5
# speedup vs baseline: 2.5142x; 1.0229x over previous
"""Trainium2 Bass kernel for nn_CompetitiveNetwork (competitive-binding solve).

Math (per batch column):
    K  = clip(exp(K_raw), 0, 1e3)   BT = clip(exp(BT_raw), 0, 1e3)
    fixed point:  u = 1/(1 + K^T AF);  AF = AT / (1 + (K diag(BT)) u)
    readout:      Y = sum_b (M^T AF)_b * u_b + b,  M = K*W*BT

Device algorithm (accelerated, validated numerically on host):
  state G (gain; AF = AT*G) and u. Per iteration:
    S  = W1blk @ af          (fp32r matmul, 128-part block-diag = 2 streams)
    u  = 1/(1+S)             (ACT reciprocal, exact)
    Tw = wg*W2blk @ u        (fp32r matmul; wg folded into weights)
    G  = ((1+wg) - (Tw+wg))*G)*G   one fused DVE op == SOR(wg) + Newton
         (iters 1..E_G use the exact path: ACT recip + scalar_tensor_tensor)
    af = AT*G                (DVE/Pool mul)
  9 over-relaxed iterations (wg=1.4) replace the reference's 21.5 plain
  iterations: SOR contraction ~0.33/iter vs 0.6, landing ~1.8e-3 from the
  reference iterate (fixed-point limit itself is only ~9e-5 away).
  All matmuls use fp32r (1 cyc/row at FD>=256; measured 9e-5 accurate on hw).

Sharding: pure data-parallel over batch (16384 -> 8 cores x 2048).
Layout: features on partitions, batch on free dim; two 64-partition streams
stacked into (128, FD) tiles; 4 column chunks of FD=256.
"""

import numpy as np

import concourse.bacc as bacc
import concourse.mybir as mybir
from concourse.tile import TileContext
from concourse.bass_utils import run_bass_kernel_spmd


# --- custom DVE op: NEWTON1P_ANT ---
# out = (c1 - (in0 + c0) * in1) * in1 : with (c0,c1)=(1,2) one Newton step
# of in1 toward 1/(1+in0); with (c0,c1)=(w,1+w) and in0=w*T it fuses the
# Newton step with SOR mixing: out = (1-w)*in1 + w*newton(in1; 1+T).

import concourse.dve_ops as dve_ops
from concourse.dve_ops import DveOp
from concourse.dve_spec import Spec, Src0, Src1, C0, C1, lower


def _ref_newton1p(in0, in1, c0, c1, c2):
    return ((c1 - (in0.astype(np.float32) + c0) * in1) * in1).astype(np.float32)


def _make_op(shas):
    return DveOp(
        "NEWTON1P_ANT",
        Spec(
            body=(C1 - (Src0 + C0) * Src1) * Src1,
            reference=_ref_newton1p,
        ),
        subdim=False,
        uops_sha=shas,
    )


def register():
    for op in dve_ops.OPS:
        if op.name == "NEWTON1P_ANT":
            return op
    probe = _make_op({})
    opcode = dve_ops._CUSTOM_DVE_ROW_BASE + len(dve_ops.OPS)
    shas = {}
    for ver in ("v3", "v4"):
        try:
            from concourse.dve_uop import DveOpSpec
            res = DveOpSpec(name=probe.name, opcode=opcode,
                            uops=lower(probe.spec, ver=ver),
                            rd1_en=True)
            shas[ver] = res.sha(ver)
        except Exception as e:
            print(f"lower {ver} failed: {e}")
    op = _make_op(shas)
    dve_ops.OPS.append(op)
    dve_ops.CUSTOM_DVE_SPECS[op.name] = op.spec
    dve_ops._SUB_OPCODE_FOR_NAME[op.name] = (
        dve_ops._CUSTOM_DVE_ROW_BASE + len(dve_ops.OPS) - 1)
    return op


def newton_sor(nc_vector, out, in0, in1, c0, c1):
    """out = (c1 - (in0 + c0) * in1) * in1 on the DVE."""
    op = register()
    return nc_vector._custom_dve(op, out=out, in0=in0, in1=in1,
                                 s0=float(c0), s1=float(c1), imm2=0.0)


B, NA, NB = 16384, 64, 64
N_CORES = 8
B_CORE = B // N_CORES          # 2048 batch columns per core
N_CHUNK = 4
FD = B_CORE // 2 // N_CHUNK    # 256

N_ITERS = 9                    # over-relaxed iterations
WG = 1.4                       # SOR factor on the G (gain) update
E_G = 3                        # iters 1..E_G use the exact ACT+stt G path

FP32 = mybir.dt.float32
FP32R = mybir.dt.float32r

_CACHE = {}


def _act_recip(nc, out_ap, in_ap, bias=1.0, scale=1.0):
    """out = 1/(in*scale + bias) on the Activation engine."""
    eng = nc.scalar
    ins = [eng.lower_ap(in_ap),
           mybir.ImmediateValue(dtype=FP32, value=float(bias)),
           mybir.ImmediateValue(dtype=FP32, value=float(scale)),
           mybir.ImmediateValue(dtype=FP32, value=0.0)]
    eng.add_instruction(mybir.InstActivation(
        name=nc.get_next_instruction_name(),
        func=mybir.ActivationFunctionType.Reciprocal,
        ins=ins, outs=[eng.lower_ap(out_ap)]))


def _build_module(repeat=1):
    register()
    nc = bacc.Bacc()
    att = nc.dram_tensor("att", (128, N_CHUNK * FD), FP32, kind="ExternalInput")
    # weights: [W1blk | W2a | W2b | M2blk | ones2]  (128, 514)
    wts = nc.dram_tensor("wts", (128, 4 * 128 + 2), FP32, kind="ExternalInput")
    yout = nc.dram_tensor("yout", (2 * N_CHUNK, FD), FP32, kind="ExternalOutput")

    with TileContext(nc) as tc, \
         tc.tile_pool(name="const", bufs=1) as cpool, \
         tc.tile_pool(name="state", bufs=2) as spool, \
         tc.tile_pool(name="work", bufs=3) as wpool, \
         tc.tile_pool(name="psum", bufs=8, space="PSUM") as ppool:

        wall = cpool.tile([128, 4 * 128 + 2], FP32, tag="wall")
        nc.sync.dma_start(out=wall[:], in_=wts[:, :])
        wallr = cpool.tile([128, 4 * 128 + 2], FP32R, tag="wallr")
        nc.vector.tensor_copy(wallr[:], wall[:])
        w1r = wallr[:, 0:128]
        w2ar = wallr[:, 128:256]
        w2br = wallr[:, 256:384]
        m2r = wallr[:, 384:512]
        onesr = wallr[:, 512:514]

        ats, atrs = [], []
        for c in range(N_CHUNK):
            at_c = cpool.tile([128, FD], FP32, tag=f"at{c}")
            nc.sync.dma_start(out=at_c[:], in_=att[:, c * FD:(c + 1) * FD])
            ats.append(at_c)
            atr_c = cpool.tile([128, FD], FP32R, tag=f"atr{c}")
            nc.vector.tensor_copy(atr_c[:], at_c[:])
            atrs.append(atr_c)

        for _rep in range(repeat):
            af = list(atrs)             # AF_0 = AT (G_0 = 1)
            us = [None] * N_CHUNK
            gs = [None] * N_CHUNK
            pss = [None] * N_CHUNK
            ps2s = [None] * N_CHUNK

            def emit_halfstep(c, h):
                n = h // 2 + 1          # iteration number, 1-based
                if h % 2 == 0:
                    # S-side: ps = W1 af ; u = 1/(1+S) on ACT
                    ps = ppool.tile([128, FD], FP32, tag="ps")
                    nc.tensor.matmul(out=ps[:], lhsT=w1r, rhs=af[c][:],
                                     start=True, stop=True)
                    pss[c] = ps
                    u_n = spool.tile([128, FD], FP32R, tag=f"u{c}")
                    _act_recip(nc, u_n[:], ps[:])
                    us[c] = u_n
                else:
                    # T-side: ps2 = wg*W2 u ; G update ; af = AT*G
                    ps2 = ppool.tile([128, FD], FP32, tag="ps")
                    nc.tensor.matmul(out=ps2[:], lhsT=(w2ar if n == 1 else w2br),
                                     rhs=us[c][:], start=True, stop=True)
                    ps2s[c] = ps2
                    g_n = spool.tile([128, FD], FP32, tag=f"g{c}")
                    if n == 1:
                        # G_1 = 1/(1+T)   (wg=1 first step)
                        _act_recip(nc, g_n[:], ps2[:])
                    elif n <= E_G:
                        # exact SOR: ACT emits R = wg/(1+T) directly
                        # (1/(ps2*s + b) with s=1/wg^2, b=1/wg on ps2=wg*T),
                        # then G = (1-wg)*G + R in one stt.
                        r_n = wpool.tile([128, FD], FP32, tag=f"r{c}")
                        _act_recip(nc, r_n[:], ps2[:],
                                   bias=1.0 / WG, scale=1.0 / (WG * WG))
                        nc.vector.scalar_tensor_tensor(
                            out=g_n[:], in0=gs[c][:], scalar=float(1.0 - WG),
                            in1=r_n[:], op0=mybir.AluOpType.mult,
                            op1=mybir.AluOpType.add)
                    else:
                        # fused Newton+SOR: G = ((1+wg)-(ps2+wg)*G)*G
                        newton_sor(nc.vector, g_n[:], ps2[:], gs[c][:],
                                   WG, 1.0 + WG)
                    gs[c] = g_n
                    af_n = spool.tile([128, FD], FP32R, tag=f"af{c}")
                    if c < 2:
                        nc.vector.tensor_mul(af_n[:], ats[c][:], gs[c][:])
                    else:
                        nc.gpsimd.tensor_mul(af_n[:], ats[c][:], gs[c][:])
                    af[c] = af_n

            # group B (chunks 2,3) runs one half-step behind group A (0,1):
            # each tick mixes S-side and T-side work so every engine's
            # in-order stream has a steady supply of ready instructions.
            H = 2 * N_ITERS
            for t in range(H + 1):
                for c in (0, 1):
                    if t < H:
                        emit_halfstep(c, t)
                for c in (2, 3):
                    if t >= 1:
                        emit_halfstep(c, t - 1)

            # readout: S = W1 af; u = newton(u); V = M2 af; h = V*u; Y = ones^T h
            for c in range(N_CHUNK):
                ps = ppool.tile([128, FD], FP32, tag="ps")
                nc.tensor.matmul(out=ps[:], lhsT=w1r, rhs=af[c][:],
                                 start=True, stop=True)
                pp = ppool.tile([128, FD], FP32, tag="ps")
                nc.tensor.matmul(out=pp[:], lhsT=m2r, rhs=af[c][:],
                                 start=True, stop=True)
                u1 = spool.tile([128, FD], FP32, tag=f"u{c}")
                newton_sor(nc.vector, u1[:], ps[:], us[c][:], 1.0, 2.0)
                u2 = spool.tile([128, FD], FP32, tag=f"uf{c}")
                newton_sor(nc.vector, u2[:], ps[:], u1[:], 1.0, 2.0)
                h = wpool.tile([128, FD], FP32R, tag="h")
                nc.vector.tensor_mul(h[:], pp[:], u2[:])
                yp = ppool.tile([128, FD], FP32, tag="ps")
                nc.tensor.matmul(out=yp[0:2, :], lhsT=onesr,
                                 rhs=h[:], start=True, stop=True)
                ys = wpool.tile([128, FD], FP32, tag="ys")
                nc.scalar.copy(ys[0:2, :], yp[0:2, :])
                nc.sync.dma_start(out=yout[2 * c:2 * c + 2, :], in_=ys[0:2, :])

    nc.finalize()
    return nc


def _get_module(repeat=1):
    key = f"nc{repeat}"
    if key not in _CACHE:
        _CACHE[key] = _build_module(repeat)
    return _CACHE[key]


def kernel(AT, K_raw, BT_raw, W_raw, b_raw, _run_kw=None, _repeat=1):
    AT = np.asarray(AT, dtype=np.float32)
    K = np.clip(np.exp(np.asarray(K_raw, np.float32)), 0.0, 1000.0).astype(np.float32)
    BT = np.clip(np.exp(np.asarray(BT_raw, np.float32)), 0.0, 1000.0).astype(np.float32)
    Wc = np.clip(np.asarray(W_raw, np.float32), -10.0, 10.0).reshape(NA, NB)
    b0 = np.clip(np.asarray(b_raw, np.float32), -10.0, 10.0)[0]

    w2 = np.ascontiguousarray((K * BT[None, :]).T)     # (nB,nA) lhsT: T = w2^T u
    M = K * Wc * BT[None, :]                           # (nA,nB) lhsT: V = M^T af

    def blk(a):
        z = np.zeros((128, 128), np.float32)
        z[0:64, 0:64] = a
        z[64:128, 64:128] = a
        return z

    ones2 = np.zeros((128, 2), np.float32)
    ones2[0:64, 0] = 1.0
    ones2[64:128, 1] = 1.0
    wts = np.ascontiguousarray(np.concatenate(
        [blk(K), blk(w2), blk(WG * w2), blk(M), ones2], axis=1))

    att = np.ascontiguousarray(AT.T)                   # (64, 16384)
    HB = B_CORE // 2                                   # 1024 cols per stream
    in_maps = []
    for c in range(N_CORES):
        chunk = att[:, c * B_CORE:(c + 1) * B_CORE]    # (64, 2048)
        stacked = np.ascontiguousarray(
            np.concatenate([chunk[:, :HB], chunk[:, HB:]], axis=0))
        in_maps.append({"att": stacked, "wts": wts})

    nc = _get_module(_repeat)
    res = run_bass_kernel_spmd(nc, in_maps, core_ids=list(range(N_CORES)),
                               **(_run_kw or {}))
    out = np.empty((B,), np.float32)
    for co in range(N_CORES):
        y = res.results[co]["yout"]                    # (8, 256)
        base = co * B_CORE
        for c in range(N_CHUNK):
            out[base + c * FD:base + (c + 1) * FD] = y[2 * c]
            out[base + HB + c * FD:base + HB + (c + 1) * FD] = y[2 * c + 1]
    if _run_kw is not None:
        _CACHE["last_result"] = res
    return out + b0
